# revision 8
# baseline (speedup 1.0000x reference)
"""Causal cross-attention Trainium2 Bass kernel.

Problem: nn_CausalCrossAttention (E=1024, H=16, HD=64, L=1024, S=1025, B=8).
Sharding: data-parallel over batch -- one batch element per NeuronCore (8 cores).

Per-core device program (all matmuls bf16):
  1. QP^T[e',l] / KP^T[e',s] projections (feature-major, from host-pretransposed
     weights), VP[s,e'] natural, interleaved into VPE with a per-head "16.0"
     column (gives 16*softmax-sum for free in the ctx matmul).
  2. Per head: scores^T[s,l] = KP_h^T.T @ QP_h^T (causal-sparse, head pairs run
     concurrently on PE row groups), exp on ScalarE (scale=1/8), multiplicative
     causal mask, ctx_ext^T = VPE_h.T @ exp^T, reciprocal of the sums row,
     rank-1 PE broadcast of 1/(16S) across partitions (fp32r), normalized ctx
     and head-mean accumulation (pair tree in bf16, two accumulators).
  3. out = CTXN.T @ (16*out_w.T); attn-mean blocks PE-transposed back to [l,s]
     natural layout; masked region exact 0.
"""

import numpy as np
import ml_dtypes

import concourse.bass as bass
import concourse.bacc as bacc
import concourse.mybir as mybir
import concourse.tile as tile
from concourse import bass_utils

E = 1024
H = 16
HD = 64
L = 1024
S = 1025
B = 8
P = 128
NT = E // P          # 8 e-tiles
LT = L // P          # 8 l-tiles
ST = 9               # s-tiles, last has 1 row
SROWS = [P] * 8 + [1]
# sparse l-range start per s-tile c (attention allows s <= l+1  =>  l >= s-1)
L0 = [0] + [128 * c - 1 for c in range(1, 9)]
W = [1024 - l0 for l0 in L0]                        # sparse widths
AL0 = [0] + [128 * (c - 1) for c in range(1, 9)]    # 128-aligned starts
AW = [1024 - a for a in AL0]                        # aligned widths
AOFF = [L0[c] - AL0[c] for c in range(9)]           # 0 or 127

BF = mybir.dt.bfloat16
F32 = mybir.dt.float32
F32R = mybir.dt.float32r
EXP = mybir.ActivationFunctionType.Exp

_CACHE = {}


def build_nc():
    nc = bacc.Bacc("TRN2", target_bir_lowering=False, debug=False)

    qT = nc.dram_tensor("qT", [E, L], BF, kind="ExternalInput")
    cT = nc.dram_tensor("cT", [E, S], BF, kind="ExternalInput")
    wqT = nc.dram_tensor("wqT", [E, E], BF, kind="ExternalInput")
    wkT = nc.dram_tensor("wkT", [E, E], BF, kind="ExternalInput")
    wvT = nc.dram_tensor("wvT", [E, E], BF, kind="ExternalInput")
    owT = nc.dram_tensor("owT", [E, E], BF, kind="ExternalInput")
    msk = nc.dram_tensor("msk", [P, 2 * P], BF, kind="ExternalInput")  # [M0|M1]
    ones = nc.dram_tensor("ones", [P, P], BF, kind="ExternalInput")
    iden = nc.dram_tensor("iden", [P, P], BF, kind="ExternalInput")
    out = nc.dram_tensor("out", [L, E], F32, kind="ExternalOutput")
    am = nc.dram_tensor("am", [L, S], F32, kind="ExternalOutput")

    with tile.TileContext(nc) as tc:
        build_body(nc, tc, qT, cT, wqT, wkT, wvT, owT, msk, ones, iden, out, am)
    nc.finalize()
    return nc


def build_body(nc, tc, qT, cT, wqT, wkT, wvT, owT, msk, ones, iden, out, am):
    from contextlib import ExitStack

    with ExitStack() as ctx:
        cpool = ctx.enter_context(tc.tile_pool(name="const", bufs=1))
        tmsk = cpool.tile([P, 2 * P], BF, tag="msk", name="msk")
        nc.sync.dma_start(tmsk[:], msk[:, :])
        tones = cpool.tile([P, P], BF, tag="ones", name="ones")
        nc.sync.dma_start(tones[:], ones[:, :])
        tiden = cpool.tile([P, P], BF, tag="iden", name="iden")
        nc.sync.dma_start(tiden[:], iden[:, :])

        # persistent activations
        qkv = ctx.enter_context(tc.tile_pool(name="qkv", bufs=1))
        QPT = [qkv.tile([P, L], BF, tag=f"qpt{t}", name=f"qpt{t}") for t in range(NT)]
        KPT = [qkv.tile([P, S], BF, tag=f"kpt{t}", name=f"kpt{t}") for t in range(NT)]
        VPE = [qkv.tile([SROWS[c], 16 * 65], BF, tag=f"vpe{c}", name=f"vpe{c}") for c in range(ST)]

        # ---------------- phase 1: projections ----------------
        with ExitStack() as pctx:
            inp = pctx.enter_context(tc.tile_pool(name="pin", bufs=1))
            qTs = [inp.tile([P, L], BF, tag=f"qts{t}", name=f"qts{t}") for t in range(NT)]
            cTs = [inp.tile([P, S], BF, tag=f"cts{t}", name=f"cts{t}") for t in range(NT)]
            for t in range(NT):
                nc.sync.dma_start(qTs[t][:], qT[P * t:P * (t + 1), :])
                nc.sync.dma_start(cTs[t][:], cT[P * t:P * (t + 1), :])

            wpool = pctx.enter_context(tc.tile_pool(name="w", bufs=2))
            ppj = pctx.enter_context(tc.tile_pool(name="ppj", bufs=4, space="PSUM"))

            # ones columns of VPE: value 16.0 (the /16 of the head-mean is
            # folded here; out_w is host-scaled by 16 to compensate on ctx)
            for c in range(ST):
                vap = VPE[c][:].rearrange("p (h d) -> p h d", d=65)
                nc.gpsimd.memset(vap[:, :, 64:65], 16.0)

            # Q projection: QPT[m] = sum_t wqT[t][:, m].T @ qTs[t]
            wt = [wpool.tile([P, E], BF, tag=f"w{t}", name=f"w{t}") for t in range(NT)]
            for t in range(NT):
                nc.sync.dma_start(wt[t][:], wqT[P * t:P * (t + 1), :])
            for m in range(NT):
                for k in range(2):
                    ps = ppj.tile([P, 512], F32, tag="ppj", name="ppj")
                    for t in range(NT):
                        nc.tensor.matmul(ps[:], wt[t][:, P * m:P * (m + 1)],
                                         qTs[t][:, 512 * k:512 * (k + 1)],
                                         start=(t == 0), stop=(t == NT - 1))
                    nc.vector.tensor_copy(QPT[m][:, 512 * k:512 * (k + 1)], ps[:])

            # K projection (S = 1025 -> chunks 512,512,1)
            wt = [wpool.tile([P, E], BF, tag=f"w{t}", name=f"w{t}") for t in range(NT)]
            for t in range(NT):
                nc.sync.dma_start(wt[t][:], wkT[P * t:P * (t + 1), :])
            for m in range(NT):
                for n0, n1 in [(0, 512), (512, 1024), (1024, 1025)]:
                    ps = ppj.tile([P, 512], F32, tag="ppj", name="ppj")
                    for t in range(NT):
                        nc.tensor.matmul(ps[:, 0:n1 - n0], wt[t][:, P * m:P * (m + 1)],
                                         cTs[t][:, n0:n1],
                                         start=(t == 0), stop=(t == NT - 1))
                    nc.vector.tensor_copy(KPT[m][:, n0:n1], ps[:, 0:n1 - n0])

            # V projection, natural [s, e'], interleaved into VPE (stride 65)
            wt = [wpool.tile([P, E], BF, tag=f"w{t}", name=f"w{t}") for t in range(NT)]
            for t in range(NT):
                nc.sync.dma_start(wt[t][:], wvT[P * t:P * (t + 1), :])
            for c in range(ST):
                rows = SROWS[c]
                for k in range(2):
                    ps = ppj.tile([P, 512], F32, tag="ppj", name="ppj")
                    for t in range(NT):
                        nc.tensor.matmul(ps[0:rows, :],
                                         cTs[t][:, 128 * c:128 * c + rows],
                                         wt[t][:, 512 * k:512 * (k + 1)],
                                         start=(t == 0), stop=(t == NT - 1))
                    vap = VPE[c][:].rearrange("p (h d) -> p h d", d=65)
                    pap = ps[0:rows, :].rearrange("p (h d) -> p h d", d=64)
                    nc.vector.tensor_copy(vap[:, 8 * k:8 * (k + 1), 0:64], pap[:])

        # ---------------- phase 2: per-head attention ----------------
        apool = ctx.enter_context(tc.tile_pool(name="attn", bufs=1))
        # bf16 head-mean accumulators (A1: heads 0..7, A2: heads 8..15)
        A1 = [apool.tile([SROWS[c], AW[c]], BF, tag=f"a1_{c}", name=f"a1_{c}") for c in range(ST)]
        A2 = [apool.tile([SROWS[c], AW[c]], BF, tag=f"a2_{c}", name=f"a2_{c}") for c in range(ST)]
        for c in range(1, ST):
            nc.gpsimd.memset(A1[c][:, 0:AOFF[c]], 0.0)
            nc.gpsimd.memset(A2[c][:, 0:AOFF[c]], 0.0)
        CTXN = [apool.tile([P, L], BF, tag=f"ctxn{t}", name=f"ctxn{t}") for t in range(8)]

        with ExitStack() as actx:
            epool = actx.enter_context(tc.tile_pool(name="expt", bufs=1))
            nepool = actx.enter_context(tc.tile_pool(name="ne", bufs=1))
            rpool = actx.enter_context(tc.tile_pool(name="rsr", bufs=2))
            rbpool = actx.enter_context(tc.tile_pool(name="rbp", bufs=3))
            psc = actx.enter_context(tc.tile_pool(name="psc", bufs=2, space="PSUM"))
            pctxp = actx.enter_context(tc.tile_pool(name="pctx", bufs=2, space="PSUM"))
            prb = actx.enter_context(tc.tile_pool(name="prb", bufs=1, space="PSUM"))

            cmpool = actx.enter_context(tc.tile_pool(name="cm", bufs=2))
            for pair in range(8):
                pd = {}
                CTXM = cmpool.tile([64, 2 * 1024], BF, tag="ctxm", name="ctxm")
                for h in (2 * pair, 2 * pair + 1):
                    off = (h % 2) * 64
                    t = h // 2
                    expt = {}
                    for c in range(ST):
                        rows, w, l0 = SROWS[c], W[c], L0[c]
                        ps = psc.tile([P, 1024], F32, tag="psc", name="psc")
                        # scores^T[s, l] = KP_h^T.T @ QP_h^T, K = 64 head dims
                        for n0 in range(0, w, 512):
                            n1 = min(n0 + 512, w)
                            nc.tensor.matmul(
                                ps[0:rows, n0:n1],
                                KPT[t][off:off + 64, 128 * c:128 * c + rows],
                                QPT[t][off:off + 64, l0 + n0:l0 + n1],
                                start=True, stop=True)
                        et = epool.tile([rows, w], BF, tag=f"expt{c}_{h % 2}", name=f"expt{c}_{h % 2}")
                        nc.scalar.activation(et[:], ps[0:rows, 0:w], EXP,
                                             bias=0.0, scale=0.125)
                        if c < 8:
                            # multiplicative causal mask on first 128 cols
                            mt = tmsk[0:rows, 0:P] if c == 0 else tmsk[0:rows, P:2 * P]
                            nc.vector.tensor_mul(et[:, 0:P], et[:, 0:P], mt)
                        expt[c] = et

                    # ctx_ext^T accumulation into psum [65, 512] per l-chunk
                    rsr = rpool.tile([65, 1024], BF, tag="rsr", name="rsr")
                    rbps = prb.tile([P, 1024], F32, tag="prb", name="prb")
                    rb = rbpool.tile([P, 1024], BF, tag="rb", name="rb")
                    pcs = []
                    for k in range(2):
                        lo, hi = 512 * k, 512 * (k + 1)
                        cs = [c for c in range(ST) if L0[c] < hi]
                        pc = pctxp.tile([65, 512], F32, tag="pctx", name="pctx")
                        for i, c in enumerate(cs):
                            start = max(L0[c], lo)
                            nc.tensor.matmul(
                                pc[:, start - lo:hi - lo],
                                VPE[c][:, 65 * h:65 * h + 65],
                                expt[c][:, start - l0c(c):hi - l0c(c)],
                                start=(i == 0), stop=(i == len(cs) - 1))
                        # reciprocal of 16*rowsum (psum row 64, lane-locked p64)
                        with nc.allow_low_precision(reason="bf16 softmax recip"):
                            nc.vector.reciprocal(rsr[64:65, lo:hi], pc[64:65, :])
                        # rank-1 broadcast to all 128 partitions (bf16 matmul)
                        nc.tensor.matmul(rbps[:, lo:hi],
                                         tones[64:65, :],
                                         rsr[64:65, lo:hi],
                                         start=True, stop=True)
                        pcs.append(pc)
                    nc.scalar.copy(rb[:], rbps[:])
                    # normalized ctx slices -> CTXM (psum * sbuf-bf16 recip)
                    for k in range(2):
                        lo, hi = 512 * k, 512 * (k + 1)
                        nc.vector.tensor_mul(
                            CTXM[0:64, 1024 * (h % 2) + lo:1024 * (h % 2) + hi],
                            pcs[k][0:64, :], rb[0:64, lo:hi])
                    # head-mean partial: NE = exp * bcast-recip, pair tree
                    for c in range(ST):
                        rows, w, l0 = SROWS[c], W[c], L0[c]
                        ne = nepool.tile([rows, w], BF, tag=f"ne{c}_{h % 2}", name=f"ne{c}_{h % 2}")
                        nc.vector.tensor_mul(ne[:], expt[c][:], rb[0:rows, l0:1024])
                        if h % 2 == 0:
                            pd[c] = ne
                        else:
                            A = A1 if pair < 4 else A2
                            if pair % 4 == 0:
                                nc.vector.tensor_add(A[c][:, AOFF[c]:], pd[c][:], ne[:])
                            else:
                                nc.vector.tensor_add(pd[c][:], pd[c][:], ne[:])
                                nc.vector.tensor_add(A[c][:, AOFF[c]:],
                                                     A[c][:, AOFF[c]:], pd[c][:])
                    # restack normalized ctx into e-major CTXN tiles
                    nc.sync.dma_start(
                        CTXN[h // 2][(h % 2) * 64:(h % 2) * 64 + 64, :],
                        CTXM[0:64, 1024 * (h % 2):1024 * (h % 2 + 1)])

        # ---------------- phase 3: outputs ----------------
        with ExitStack() as octx:
            wpool2 = octx.enter_context(tc.tile_pool(name="w2", bufs=1))
            opool = octx.enter_context(tc.tile_pool(name="outs", bufs=2))
            ppo = octx.enter_context(tc.tile_pool(name="ppo", bufs=4, space="PSUM"))
            ow = [wpool2.tile([P, E], BF, tag=f"ow{t}", name=f"ow{t}") for t in range(NT)]
            for t in range(NT):
                nc.sync.dma_start(ow[t][:], owT[P * t:P * (t + 1), :])
            for m in range(LT):
                ot = opool.tile([P, E], F32, tag="outs", name="outs")
                for k in range(2):
                    ps = ppo.tile([P, 512], F32, tag="ppo", name="ppo")
                    for t in range(NT):
                        nc.tensor.matmul(ps[:], CTXN[t][:, P * m:P * (m + 1)],
                                         ow[t][:, 512 * k:512 * (k + 1)],
                                         start=(t == 0), stop=(t == NT - 1))
                    nc.vector.tensor_copy(ot[:, 512 * k:512 * (k + 1)], ps[:])
                nc.sync.dma_start(out[P * m:P * (m + 1), :], ot[:])

            # attn-mean: AS = A1 + A2, transpose blocks back to [l, s], DMA out
            arow = octx.enter_context(tc.tile_pool(name="arow", bufs=2))
            ptr = octx.enter_context(tc.tile_pool(name="ptr", bufs=4, space="PSUM"))
            aspool = octx.enter_context(tc.tile_pool(name="as", bufs=1))
            AS = [aspool.tile([SROWS[c], AW[c]], BF, tag=f"as{c}", name=f"as{c}")
                  for c in range(ST)]
            for c in range(ST):
                nc.vector.tensor_add(AS[c][:], A1[c][:], A2[c][:])
            for k in range(LT):
                at = arow.tile([P, S], F32, tag="arow", name="arow")
                ncols = 128 * (k + 2)
                if ncols < S:
                    nc.gpsimd.memset(at[:, ncols:S], 0.0)
                for c in range(0, min(k + 1, 8) + 1):
                    j = k if c == 0 else k - (c - 1)
                    rows = SROWS[c]
                    t1 = ptr.tile([P, P], BF, tag="ptr", name="ptr")
                    nc.tensor.transpose(t1[0:P, 0:rows],
                                        AS[c][:, 128 * j:128 * (j + 1)],
                                        tiden[0:rows, 0:rows])
                    wid = min(128 * c + rows, S) - 128 * c
                    nc.vector.tensor_copy(at[:, 128 * c:128 * c + wid],
                                          t1[0:P, 0:wid])
                nc.sync.dma_start(am[P * k:P * (k + 1), :], at[:])


def l0c(c):
    return L0[c]


def _marshal(q, cond, in_proj_w, out_w):
    bf = ml_dtypes.bfloat16
    wqT = np.ascontiguousarray(in_proj_w[:E].T).astype(bf)
    wkT = np.ascontiguousarray(in_proj_w[E:2 * E].T).astype(bf)
    wvT = np.ascontiguousarray(in_proj_w[2 * E:].T).astype(bf)
    owT = np.ascontiguousarray(out_w.T * 16.0).astype(bf)

    i = np.arange(P)[:, None]
    j = np.arange(P)[None, :]
    m0 = (j >= i - 1).astype(bf)
    m1 = (j >= i).astype(bf)
    msk = np.concatenate([m0, m1], axis=1)
    ones = np.ones((P, P), dtype=bf)
    iden = np.eye(P, dtype=np.float32).astype(bf)

    in_maps = []
    for b in range(B):
        in_maps.append({
            "qT": np.ascontiguousarray(q[:, b, :].T).astype(bf),
            "cT": np.ascontiguousarray(cond[:, b, :].T).astype(bf),
            "wqT": wqT, "wkT": wkT, "wvT": wvT, "owT": owT,
            "msk": msk, "ones": ones, "iden": iden,
        })
    return in_maps


def kernel(q, cond, in_proj_w, in_proj_b, out_w, out_b):
    q = np.asarray(q, dtype=np.float32)
    cond = np.asarray(cond, dtype=np.float32)
    in_proj_w = np.asarray(in_proj_w, dtype=np.float32)
    in_proj_b = np.asarray(in_proj_b, dtype=np.float32)
    out_w = np.asarray(out_w, dtype=np.float32)
    out_b = np.asarray(out_b, dtype=np.float32)
    assert np.all(in_proj_b == 0.0), "nonzero in_proj bias not supported"

    if "nc" not in _CACHE:
        _CACHE["nc"] = build_nc()
    nc = _CACHE["nc"]

    in_maps = _marshal(q, cond, in_proj_w, out_w)
    res = bass_utils.run_bass_kernel_spmd(nc, in_maps, core_ids=list(range(B)))

    out = np.empty((L, B, E), dtype=np.float32)
    attn = np.empty((B, L, S), dtype=np.float32)
    for b in range(B):
        out[:, b, :] = np.asarray(res.results[b]["out"], dtype=np.float32)
        attn[b] = np.asarray(res.results[b]["am"], dtype=np.float32)
    out += out_b[None, None, :]
    return out, attn
